# revision 1
# baseline (speedup 1.0000x reference)
"""GAT (3-layer, multi-head) forward on 8 Trainium2 NeuronCores.

Strategy (graph/data parallel, per sharding hint):
- Nodes sharded in contiguous 1280-node blocks (10 tiles of 128) per core;
  edges partitioned by destination, sorted by dst, padded per-tile.
- Per layer: node-phase projection (PE matmul, x^T as stationary operand),
  table rows [h | al_src hi | al_src lo | pad] written to HBM, AllGather.
- Edge phase per dst-tile: dma_gather of source rows (the only per-edge
  data movement), al_dst expanded edge-wise via one-hot^T matmul on PE,
  attention weights exp(leakyrelu(z)) on ACT, applied to messages in-place
  on DVE (broadcast AP), aggregation + softmax denominators via one-hot
  matmul accumulated in PSUM, then normalize/bias/ELU/residual.
- Graph mean-pool via one-hot matmul accumulated over tiles, AllReduce,
  classifier + log_softmax computed redundantly on every core.
"""
import sys

sys.path.insert(0, '/opt/trn_rl_repo')

import numpy as np

N_CORES = 8
N, E, F_IN, HID, H1, H3, NC_CLS, G = 10000, 160000, 256, 128, 5, 3, 10, 64
NEG_SLOPE = 0.2
SHARD = 1280            # nodes per core (10 tiles of 128); core 7 partially padded
NTILE = 10              # dst tiles per core
NPAD = SHARD * N_CORES  # 10240 padded node count
ROW = 768               # table row elems (bf16): [h 640 | al_s hi 5 | al_s lo 5 | pad]
ROW3 = 768              # layer-3 table row: [h 384 | al hi 3 | al lo 3 | pad]


def _blockdiag_a(a):
    # a: [H, C] -> [H*C, H] block diagonal so that (h @ A)[n, h] = sum_c h[n,h,c]*a[h,c]
    Hh, C = a.shape
    out = np.zeros((Hh * C, Hh), np.float32)
    for h in range(Hh):
        out[h * C:(h + 1) * C, h] = a[h]
    return out


def _hilo(x):
    import ml_dtypes
    hi = x.astype(ml_dtypes.bfloat16)
    lo = (x - hi.astype(np.float32)).astype(ml_dtypes.bfloat16)
    return hi, lo


def _build_host_data(x, edge_index, batch, weights):
    """All index preprocessing + per-core constant inputs."""
    import ml_dtypes
    bf16 = ml_dtypes.bfloat16

    src = np.concatenate([edge_index[0], np.arange(N)]).astype(np.int64)
    dst = np.concatenate([edge_index[1], np.arange(N)]).astype(np.int64)

    order = np.argsort(dst, kind='stable')
    src, dst = src[order], dst[order]

    # per (core, local_tile) edge lists
    core_of = dst // SHARD
    tile_of = (dst % SHARD) // 128
    counts = np.zeros((N_CORES, NTILE), np.int64)
    for k in range(N_CORES):
        for t in range(NTILE):
            counts[k, t] = np.count_nonzero((core_of == k) & (tile_of == t))
    nchunks = [int(np.ceil(counts[:, t].max() / 128)) for t in range(NTILE)]
    totc = sum(nchunks)

    per_core = []
    for k in range(N_CORES):
        srcs = np.zeros(totc * 128, np.int16)
        S = np.zeros((128, totc, 128), np.float32)
        base = 0
        for t in range(NTILE):
            m = (core_of == k) & (tile_of == t)
            s_t, d_t = src[m], dst[m]
            n = len(s_t)
            cap = nchunks[t] * 128
            sl = np.zeros(cap, np.int16)
            sl[:n] = s_t.astype(np.int16)
            srcs[base * 128:(base + nchunks[t]) * 128] = sl
            dloc = (d_t % SHARD) % 128
            e = np.arange(n)
            S[e % 128, base + e // 128, dloc] = 1.0
            base += nchunks[t]
        ST = np.ascontiguousarray(S.transpose(2, 1, 0))  # [128 d, totc, 128 e]

        # idx tile layout: [128, num/16] int16, idx i at [i%16, i//16], replicated x8
        def wrap_idx(flat):
            n_ = len(flat)
            cols = n_ // 16
            t_ = np.zeros((128, cols), np.int16)
            v = flat.reshape(cols, 16).T  # [16, cols]
            for g_ in range(8):
                t_[g_ * 16:(g_ + 1) * 16, :] = v
            return t_

        # pool matrix: [128, NTILE, 64] one-hot node->graph (1.0), zero for pads
        pool = np.zeros((128, NTILE, G), np.float32)
        for t in range(NTILE):
            gn = SHARD * k + 128 * t + np.arange(128)
            ok = gn < N
            pool[ok, t, batch[gn[ok]]] = 1.0

        per_core.append(dict(
            src_idx=wrap_idx(srcs),
            S=S.astype(bf16), ST=ST.astype(bf16),
            pool=pool.astype(bf16),
        ))

    cnt = np.bincount(batch, minlength=G).astype(np.float32)
    inv_cnt = (1.0 / np.maximum(cnt, 1.0)).astype(np.float32)

    W1, a1s, a1d, b1, W2, a2s, a2d, b2, W3, a3s, a3d, b3, Wc, bc = weights
    # extended weights: [in, 640 | al_s hi/lo... ] -> al columns computed at f32,
    # but the matmul is bf16; al hi/lo split happens post-matmul on device from
    # f32 psum. Ext layout: [h (Hh*HID) | al_s (Hh) | al_d (Hh) | pad]
    def ext(W, as_, ad_, width):
        A_s = W @ _blockdiag_a(as_)
        A_d = W @ _blockdiag_a(ad_)
        out = np.zeros((W.shape[0], width), np.float32)
        out[:, :W.shape[1]] = W
        out[:, W.shape[1]:W.shape[1] + as_.shape[0]] = A_s
        out[:, W.shape[1] + as_.shape[0]:W.shape[1] + 2 * as_.shape[0]] = A_d
        return out

    W1e = ext(W1, a1s, a1d, 656).astype(bf16)   # cols: 640 h, 640:645 als, 645:650 ald
    W2e = ext(W2, a2s, a2d, 656).astype(bf16)
    W3e = ext(W3, a3s, a3d, 392).astype(bf16)   # 384 h, 384:387 als, 387:390 ald

    xt_full = np.zeros((F_IN, NPAD), np.float32)
    xt_full[:, :N] = x.T
    xt_full = xt_full.astype(bf16)

    consts = dict(
        W1e=W1e, W2e=W2e, W3e=W3e,
        b1r=np.tile(b1[None, :], (128, 1)).astype(np.float32),
        b2r=np.tile(b2[None, :], (128, 1)).astype(np.float32),
        b3r=np.tile(b3[None, :], (128, 1)).astype(np.float32),
        Wc=np.concatenate([Wc, np.zeros((HID, 6), np.float32)], 1).astype(bf16),  # [128, 16]
        bcr=np.tile(bc[None, :], (G, 1)).astype(np.float32),
        inv_cnt=inv_cnt.reshape(G, 1),
    )
    for k in range(N_CORES):
        xt_loc = np.asarray(xt_full[:, SHARD * k:SHARD * (k + 1)])
        per_core[k]['xt_loc'] = np.ascontiguousarray(xt_loc)
        per_core[k].update(consts)
    return per_core, nchunks, totc


def _build_bass(nchunks, totc):
    import concourse.bass as bass
    import concourse.mybir as mybir
    import concourse.tile as tile
    from concourse import bacc

    bf = mybir.dt.bfloat16
    f32 = mybir.dt.float32
    AT = mybir.ActivationFunctionType
    OP = mybir.AluOpType

    nc = bacc.Bacc("TRN2", target_bir_lowering=False, debug=False,
                   num_devices=N_CORES)

    # ---- inputs -----------------------------------------------------------
    d_xtl = nc.dram_tensor("xt_loc", [F_IN, SHARD], bf, kind="ExternalInput")
    d_W1e = nc.dram_tensor("W1e", [F_IN, 656], bf, kind="ExternalInput")
    d_W2e = nc.dram_tensor("W2e", [640, 656], bf, kind="ExternalInput")
    d_W3e = nc.dram_tensor("W3e", [640, 392], bf, kind="ExternalInput")
    d_b1 = nc.dram_tensor("b1r", [128, 640], f32, kind="ExternalInput")
    d_b2 = nc.dram_tensor("b2r", [128, 640], f32, kind="ExternalInput")
    d_b3 = nc.dram_tensor("b3r", [128, 128], f32, kind="ExternalInput")
    d_Wc = nc.dram_tensor("Wc", [HID, 16], bf, kind="ExternalInput")
    d_bcr = nc.dram_tensor("bcr", [G, NC_CLS], f32, kind="ExternalInput")
    d_icnt = nc.dram_tensor("inv_cnt", [G, 1], f32, kind="ExternalInput")
    d_S = nc.dram_tensor("S", [128, totc, 128], bf, kind="ExternalInput")
    d_ST = nc.dram_tensor("ST", [128, totc, 128], bf, kind="ExternalInput")
    d_idx = nc.dram_tensor("src_idx", [128, totc * 8], mybir.dt.int16, kind="ExternalInput")
    d_pool = nc.dram_tensor("pool", [128, NTILE, G], bf, kind="ExternalInput")
    d_out = nc.dram_tensor("out", [2, G, NC_CLS], f32, kind="ExternalOutput")

    H = {1: H1, 2: H1, 3: H3}
    HW = {1: 640, 2: 640, 3: 384}       # h width per layer
    NW = {1: 656, 2: 656, 3: 392}       # node-phase psum width
    cbase = [0]
    for t in range(NTILE):
        cbase.append(cbase[-1] + nchunks[t])

    with tile.TileContext(nc) as tc:
        with (
            tc.tile_pool(name="const", bufs=1) as cpool,
            tc.tile_pool(name="mm", bufs=4) as mpool,
            tc.tile_pool(name="gath", bufs=3) as gpool,
            tc.tile_pool(name="small", bufs=2) as spool,
            tc.tile_pool(name="dbgp", bufs=1) as dbgp,
            tc.tile_pool(name="psA", bufs=3, space="PSUM") as psA,
            tc.tile_pool(name="psZ", bufs=1, space="PSUM") as psZ,
            tc.tile_pool(name="psP", bufs=1, space="PSUM") as psP,
            tc.tile_pool(name="dram", bufs=1, space="DRAM") as dpool,
        ):
            # ---- resident constants --------------------------------------
            S = cpool.tile([128, totc, 128], bf)
            ST = cpool.tile([128, totc, 128], bf)
            IDX = cpool.tile([128, totc * 8], mybir.dt.int16)
            W1e = cpool.tile([128, 2, 656], bf)
            W2e = cpool.tile([128, 5, 656], bf)
            W3e = cpool.tile([128, 5, 392], bf)
            B1 = cpool.tile([128, 640], f32)
            B2 = cpool.tile([128, 640], f32)
            B3 = cpool.tile([128, 128], f32)
            WC = cpool.tile([128, 16], bf)
            BC = cpool.tile([G, NC_CLS], f32)
            ICNT = cpool.tile([G, 1], f32)
            POOLM = cpool.tile([128, NTILE, G], bf)
            X1RES = cpool.tile([128, NTILE, 640], bf)
            ALD = cpool.tile([128, NTILE, 16], bf)   # per layer overwritten: hi 0:5, lo 5:10
            nc.sync.dma_start(S[:], d_S[:])
            nc.sync.dma_start(ST[:], d_ST[:])
            nc.sync.dma_start(IDX[:], d_idx[:])
            nc.sync.dma_start(W1e[:], d_W1e[:].rearrange("(c p) w -> p c w", p=128))
            nc.sync.dma_start(W2e[:], d_W2e[:].rearrange("(c p) w -> p c w", p=128))
            nc.sync.dma_start(W3e[:], d_W3e[:].rearrange("(c p) w -> p c w", p=128))
            nc.sync.dma_start(B1[:], d_b1[:])
            nc.sync.dma_start(B2[:], d_b2[:])
            nc.sync.dma_start(B3[:], d_b3[:])
            nc.sync.dma_start(WC[:], d_Wc[:])
            nc.sync.dma_start(BC[:], d_bcr[:])
            nc.sync.dma_start(ICNT[:], d_icnt[:])
            nc.sync.dma_start(POOLM[:], d_pool[:])

            # ---- DRAM scratch / tables / collectives ---------------------
            T1s = dpool.tile([SHARD, ROW], bf)
            T1 = dpool.tile([NPAD, ROW], bf)
            T2s = dpool.tile([SHARD, ROW], bf)
            T2 = dpool.tile([NPAD, ROW], bf)
            T3s = dpool.tile([SHARD, ROW3], bf)
            T3 = dpool.tile([NPAD, ROW3], bf)
            XS = dpool.tile([SHARD, 640], bf)        # x_{l+1} scratch (reused l=1,2)
            PIN = dpool.tile([128, G], f32)          # pooled partial
            POUT = dpool.tile([128, G], f32)

            def node_tile(lhsT_ap, We, nw, hw, hs, stage_dst=None, ald_slot=None):
                """One 128-node projection tile.
                lhsT_ap: [128, nin_chunks, 128] x^T blocks; We: [128, nin_chunks, nw]
                stage_dst: (dram_ap) row-staging destination for table write
                ald_slot: t index to store al_d hi/lo into ALD
                """
                nin = We.shape[1]
                ps = psA.tile([128, 768], f32, tag="mm")
                r0 = min(512, nw)
                for c in range(nin):
                    nc.tensor.matmul(ps[:, 0:r0], lhsT_ap[:, c, :], We[:, c, 0:r0],
                                     start=(c == 0), stop=(c == nin - 1))
                if nw > 512:
                    for c in range(nin):
                        nc.tensor.matmul(ps[:, 512:nw], lhsT_ap[:, c, :],
                                         We[:, c, 512:nw],
                                         start=(c == 0), stop=(c == nin - 1))
                if stage_dst is not None:
                    row = spool.tile([128, ROW], bf, tag="rowstage")
                    nc.scalar.activation(row[:, 0:hw], ps[:, 0:hw], AT.Copy)
                    # al_s hi/lo
                    nc.vector.tensor_copy(row[:, hw:hw + hs], ps[:, hw:hw + hs])
                    lo = spool.tile([128, 8], f32, tag="lostage")
                    nc.vector.tensor_tensor(lo[:, 0:hs], ps[:, hw:hw + hs],
                                            row[:, hw:hw + hs], OP.subtract)
                    nc.vector.tensor_copy(row[:, hw + hs:hw + 2 * hs], lo[:, 0:hs])
                    nc.sync.dma_start(stage_dst, row[:])
                if ald_slot is not None:
                    t = ald_slot
                    nc.vector.tensor_copy(ALD[:, t, 0:hs], ps[:, hw + hs:hw + 2 * hs])
                    hi_w = spool.tile([128, 8], f32, tag="histage")
                    nc.vector.tensor_copy(hi_w[:, 0:hs], ALD[:, t, 0:hs])
                    lo2 = spool.tile([128, 8], f32, tag="lostage2")
                    nc.vector.tensor_tensor(lo2[:, 0:hs], ps[:, hw + hs:hw + 2 * hs],
                                            hi_w[:, 0:hs], OP.subtract)
                    nc.vector.tensor_copy(ALD[:, t, hs:2 * hs], lo2[:, 0:hs])

            def edge_phase(lyr, T, out_cb):
                """Software-pipelined: gathers lead compute by 2 halves,
                al_d-expansion matmuls lead by 1 half, so PE never stalls on
                the DVE multiply and vice versa."""
                hh = H[lyr]
                hw = HW[lyr]
                halves = []
                for t in range(NTILE):
                    nch = nchunks[t]
                    nh0 = (nch + 1) // 2
                    halves.append((t, 0, nh0, nch))
                    if nch - nh0 > 0:
                        halves.append((t, nh0, nch - nh0, nch))
                nH = len(halves)
                G_of = {}
                pz_of = {}
                pso_of = {}

                def emit_gather(i):
                    t, c0, nch_h, nch = halves[i]
                    Gt = gpool.tile([128, 9, ROW], bf, tag="G", name="Gt")
                    nidx = nch_h * 128
                    nc.gpsimd.dma_gather(
                        Gt[:, 0:nch_h, :], T[:],
                        IDX[:, (cbase[t] + c0) * 8:(cbase[t] + c0 + nch_h) * 8],
                        num_idxs=nidx, num_idxs_reg=nidx, elem_size=ROW,
                        single_packet=False)
                    G_of[i] = Gt

                def emit_pz(i):
                    t, c0, nch_h, nch = halves[i]
                    pz = psZ.tile([128, 9, 16], f32, tag="z", name="pz")
                    for c in range(nch_h):
                        nc.tensor.matmul(pz[:, c, 0:2 * hh], ST[:, cbase[t] + c0 + c, :],
                                         ALD[:, t, 0:2 * hh], start=True, stop=True)
                    pz_of[i] = pz

                def emit_compute(i):
                    t, c0, nch_h, nch = halves[i]
                    Gt = G_of.pop(i)
                    pz = pz_of.pop(i)
                    if t not in pso_of:
                        pso_of[t] = psA.tile([128, 768], f32, tag="mm", name="pso")
                    pso = pso_of[t]
                    zw = spool.tile([128, 9, 8], f32, tag="zw")
                    nc.vector.tensor_tensor(zw[:, 0:nch_h, 0:hh],
                                            Gt[:, 0:nch_h, hw:hw + hh],
                                            Gt[:, 0:nch_h, hw + hh:hw + 2 * hh], OP.add)
                    nc.vector.tensor_tensor(zw[:, 0:nch_h, 0:hh], zw[:, 0:nch_h, 0:hh],
                                            pz[:, 0:nch_h, 0:hh], OP.add)
                    nc.vector.tensor_tensor(zw[:, 0:nch_h, 0:hh], zw[:, 0:nch_h, 0:hh],
                                            pz[:, 0:nch_h, hh:2 * hh], OP.add)
                    nc.vector.scalar_tensor_tensor(zw[:, 0:nch_h, 0:hh],
                                                   zw[:, 0:nch_h, 0:hh], 0.2,
                                                   zw[:, 0:nch_h, 0:hh], OP.mult, OP.max)
                    nc.scalar.activation(Gt[:, 0:nch_h, hw:hw + hh],
                                         zw[:, 0:nch_h, 0:hh], AT.Exp)
                    g4 = Gt[:, 0:nch_h, 0:hw].rearrange("p c (h x) -> p c h x", h=hh)
                    w4 = Gt[:, 0:nch_h, hw:hw + hh].unsqueeze(-1).broadcast_to(
                        [128, nch_h, hh, HID])
                    nc.vector.tensor_tensor(g4, g4, w4, OP.mult)
                    w1 = min(512, hw + hh)
                    first = (c0 == 0)
                    last = (c0 + nch_h == nch)
                    for c in range(nch_h):
                        st = (first and c == 0)
                        sp = last and (c == nch_h - 1)
                        nc.tensor.matmul(pso[:, 0:w1], S[:, cbase[t] + c0 + c, :],
                                         Gt[:, c, 0:w1], start=st, stop=sp,
                                         skip_group_check=True)
                        if hw + hh > 512:
                            nc.tensor.matmul(pso[:, 512:hw + hh],
                                             S[:, cbase[t] + c0 + c, :],
                                             Gt[:, c, 512:hw + hh], start=st, stop=sp,
                                             skip_group_check=True)
                    if last:
                        pso = pso_of.pop(t)
                        rec = spool.tile([128, 8], f32, tag="rec")
                        nc.vector.tensor_scalar(rec[:, 0:hh], pso[:, hw:hw + hh], 1e-16,
                                                None, OP.add, OP.bypass)
                        nc.vector.reciprocal(rec[:, 0:hh], rec[:, 0:hh])
                        xt = spool.tile([128, 640], bf, tag="xt")
                        o4 = pso[:, 0:hw].rearrange("p (h x) -> p h x", h=hh)
                        r4 = rec[:, 0:hh].unsqueeze(-1).broadcast_to([128, hh, HID])
                        nc.vector.tensor_tensor(
                            xt[:, 0:hw].rearrange("p (h x) -> p h x", h=hh),
                            o4, r4, OP.mult)
                        out_cb(t, xt)

                for i in range(nH + 2):
                    if i < nH:
                        emit_gather(i)
                    if 0 <= i - 1 < nH:
                        emit_pz(i - 1)
                    if i - 2 >= 0:
                        emit_compute(i - 2)

            # =================== LAYER 1 ==================================
            # sharded projection of the local 1280-node block, then AllGather
            for t in range(NTILE):
                lx = mpool.tile([128, 2, 128], bf, tag="lx")
                nc.sync.dma_start(lx[:], d_xtl[:].rearrange("(c p) n -> p c n", p=128)
                                  [:, :, t * 128:(t + 1) * 128])
                node_tile(lx[:], W1e[:], 656, 640, H1,
                          stage_dst=T1s[t * 128:(t + 1) * 128, :], ald_slot=t)
            nc.gpsimd.collective_compute(
                "AllGather", bass.mybir.AluOpType.bypass,
                replica_groups=[list(range(N_CORES))],
                ins=[T1s.opt()], outs=[T1.opt()])

            # L1 edge phase -> x1 (resident + scratch)
            def l1_out(t, xt):
                u = spool.tile([128, 640], bf, tag="u")
                nc.vector.tensor_tensor(u[:], xt[:], B1[:], OP.add)
                m = spool.tile([128, 640], bf, tag="m")
                nc.vector.tensor_scalar(m[:], u[:], 0.0, None, OP.min, OP.bypass)
                e = spool.tile([128, 640], bf, tag="e")
                nc.scalar.activation(e[:], m[:], AT.Exp)
                nc.vector.scalar_tensor_tensor(X1RES[:, t, :], u[:], 0.0, e[:],
                                               OP.max, OP.add)
                nc.vector.tensor_scalar(X1RES[:, t, :], X1RES[:, t, :], -1.0, None,
                                        OP.add, OP.bypass)
                nc.sync.dma_start(XS[t * 128:(t + 1) * 128, :], X1RES[:, t, :])
            edge_phase(1, T1, l1_out)


            # =================== LAYER 2 ==================================
            # x1^T via dma transpose readback
            XT2 = cpool.tile([128, 5, SHARD], bf, tag="XT")
            for c in range(5):
                nc.sync.dma_start(XT2[:, c, :], XS[:, c * 128:(c + 1) * 128],
                                  transpose=True)
            for t in range(NTILE):
                node_tile(XT2[:, :, t * 128:(t + 1) * 128], W2e[:], 656, 640, H1,
                          stage_dst=T2s[t * 128:(t + 1) * 128, :], ald_slot=t)
            nc.gpsimd.collective_compute(
                "AllGather", bass.mybir.AluOpType.bypass,
                replica_groups=[list(range(N_CORES))],
                ins=[T2s.opt()], outs=[T2.opt()])

            def l2_out(t, xt):
                u = spool.tile([128, 640], bf, tag="u")
                nc.vector.tensor_tensor(u[:], xt[:], B2[:], OP.add)
                nc.vector.tensor_tensor(u[:], u[:], X1RES[:, t, :], OP.add)
                m = spool.tile([128, 640], bf, tag="m")
                nc.vector.tensor_scalar(m[:], u[:], 0.0, None, OP.min, OP.bypass)
                e = spool.tile([128, 640], bf, tag="e")
                nc.scalar.activation(e[:], m[:], AT.Exp)
                x2 = spool.tile([128, 640], bf, tag="x2")
                nc.vector.scalar_tensor_tensor(x2[:], u[:], 0.0, e[:], OP.max, OP.add)
                nc.vector.tensor_scalar(x2[:], x2[:], -1.0, None, OP.add, OP.bypass)
                nc.sync.dma_start(XS[t * 128:(t + 1) * 128, :], x2[:])
            edge_phase(2, T2, l2_out)

            # =================== LAYER 3 ==================================
            XT3 = cpool.tile([128, 5, SHARD], bf, tag="XT")
            for c in range(5):
                nc.sync.dma_start(XT3[:, c, :], XS[:, c * 128:(c + 1) * 128],
                                  transpose=True)
            for t in range(NTILE):
                node_tile(XT3[:, :, t * 128:(t + 1) * 128], W3e[:], 392, 384, H3,
                          stage_dst=T3s[t * 128:(t + 1) * 128, :], ald_slot=t)
            nc.gpsimd.collective_compute(
                "AllGather", bass.mybir.AluOpType.bypass,
                replica_groups=[list(range(N_CORES))],
                ins=[T3s.opt()], outs=[T3.opt()])

            ppool = psP.tile([128, G], f32)

            def l3_out(t, xt):
                # xt[:, 0:384] = normalized per-head out; mean over 3 heads + b3
                s = spool.tile([128, 128], f32, tag="s3")
                nc.vector.tensor_tensor(s[:], xt[:, 0:128], xt[:, 128:256], OP.add)
                nc.vector.tensor_tensor(s[:], s[:], xt[:, 256:384], OP.add)
                x3 = spool.tile([128, 128], bf, tag="x3")
                nc.vector.scalar_tensor_tensor(x3[:], s[:], 1.0 / 3.0, B3[:],
                                               OP.mult, OP.add)
                nc.tensor.matmul(ppool[:], x3[:], POOLM[:, t, :],
                                 start=(t == 0), stop=(t == NTILE - 1))
            edge_phase(3, T3, l3_out)

            # pooled partial -> AllReduce
            pp = spool.tile([128, G], f32, tag="pp")
            nc.vector.tensor_copy(pp[:], ppool[:])
            nc.sync.dma_start(PIN[:], pp[:])
            nc.gpsimd.collective_compute(
                "AllReduce", bass.mybir.AluOpType.add,
                replica_groups=[list(range(N_CORES))],
                ins=[PIN.opt()], outs=[POUT.opt()])
            pooledf = spool.tile([128, G], f32, tag="pooledf")
            nc.sync.dma_start(pooledf[:], POUT[:])
            pooledT = spool.tile([128, G], bf, tag="pooledT")
            nc.vector.tensor_copy(pooledT[:], pooledf[:])

            # classifier: logits [64 g, 16] = pooledT^T @ Wc
            psl = psZ.tile([G, 16], f32, tag="z")
            nc.tensor.matmul(psl[:], pooledT[:], WC[:], start=True, stop=True)
            lg = spool.tile([G, NC_CLS], f32, tag="lg2")
            nc.vector.tensor_scalar(lg[:], psl[:, 0:NC_CLS], ICNT[:], None,
                                    OP.mult, OP.bypass)
            nc.vector.tensor_tensor(lg[:], lg[:], BC[:], OP.add)
            # log_softmax over free dim (10)
            mx = spool.tile([G, 1], f32, tag="mx")
            nc.vector.tensor_reduce(mx[:], lg[:], mybir.AxisListType.X, OP.max)
            sh = spool.tile([G, NC_CLS], f32, tag="sh")
            nc.vector.tensor_scalar(sh[:], lg[:], mx[:], None, OP.subtract, OP.bypass)
            ex = spool.tile([G, NC_CLS], f32, tag="ex")
            nc.scalar.activation(ex[:], sh[:], AT.Exp)
            sm = spool.tile([G, 1], f32, tag="sm")
            nc.vector.tensor_reduce(sm[:], ex[:], mybir.AxisListType.X, OP.add)
            nc.scalar.activation(sm[:], sm[:], AT.Ln)
            lp = spool.tile([G, NC_CLS], f32, tag="lp")
            nc.vector.tensor_scalar(lp[:], sh[:], sm[:], None, OP.subtract, OP.bypass)
            nc.sync.dma_start(d_out[0], lg[:])
            nc.sync.dma_start(d_out[1], lp[:])

    nc.compile()
    return nc


_CACHE = {}


def kernel(**inputs):
    from concourse.bass_utils import run_bass_kernel_spmd

    x = np.asarray(inputs["x"], np.float32)
    edge_index = np.asarray(inputs["edge_index"], np.int64)
    batch = np.asarray(inputs["batch"], np.int64)
    weights = [np.asarray(inputs[k], np.float32) for k in
               ["W1", "a1s", "a1d", "b1", "W2", "a2s", "a2d", "b2",
                "W3", "a3s", "a3d", "b3", "Wc", "bc"]]

    per_core, nchunks, totc = _build_host_data(x, edge_index, batch, weights)

    key = tuple(nchunks)
    if key not in _CACHE:
        _CACHE[key] = _build_bass(nchunks, totc)
    nc = _CACHE[key]

    in_maps = [per_core[k] for k in range(N_CORES)]
    last_err = None
    out = None
    for attempt in range(10):
        try:
            res = run_bass_kernel_spmd(nc, in_maps, core_ids=list(range(N_CORES)))
            out = res.results[0]["out"]
            if np.all(np.isfinite(out)):
                return (np.asarray(out[0], np.float32),
                        np.asarray(out[1], np.float32))
        except Exception as e:  # transient NRT/device failures: retry
            last_err = e
            import time
            time.sleep(min(2 + 2 * attempt, 10))
    if out is not None:
        return np.asarray(out[0], np.float32), np.asarray(out[1], np.float32)
    raise last_err


if __name__ == "__main__":
    sys.path.insert(0, '/root/problem')
    import reference
    ins = {k: np.asarray(v) for k, v in reference.setup_inputs().items()}
    got = kernel(**ins)
    exp = reference.reference(**ins)
    for g_, e_ in zip(got, exp):
        e_ = np.asarray(e_)
        err = np.abs(g_ - e_).max() / (np.abs(e_).max() + 1e-9)
        print("rel err:", err)

